# revision 28
# baseline (speedup 1.0000x reference)
"""Trainium2 Bass kernel for GCNCriticNet (gnn_message_passing).

Graphs are 8192 independent complete graphs of 16 nodes (+ self loops): every
node has degree 16, the symmetric norm is 1/16, and GCN aggregation collapses
to a per-graph mean. Edge lists never reach the device.

Per core (16384 nodes = 1024 graphs), feature-major [128, node-cols], node
columns ordered (s, g) — node-within-graph major — so reductions and
broadcasts are fully contiguous vector ops:
  u1 = Wcomb^T Z         Z = [obs ; bcast(graph-sum obs)] (K=128 stacked)
  x1 = tanh(u1 + b1f)                                     ACT
  sx1 = group-sum(x1)    pairwise tree, DVE bf16 2x mode
  h2  = W2s^T sx1        one matmul; h2s = h2 + b2        ACT identity
  h2b = bcast(h2s)       SBUF->SBUF DMA (idle DMA engines)
  u2  = x1 + h2b         plain adds, split DVE / GPSIMD
  x2  = tanh(u2)                                          ACT
  y   = wfc^T group-sum(x2)                               DVE tree + matmul
Host: out = y + b_fc1 (mean's /16 folded into the weights).

4 macro-chunks of 4096 nodes; u1/tanh split in 1024-col quarters for PSUM.
"""

import sys
import numpy as np

try:
    import concourse.bass as bass  # noqa: F401
except ImportError:  # harness runs in a bare dir; repo is on the box
    for p in ("/opt/trn_rl_repo", "/root/.axon_site/_ro/trn_rl_repo"):
        if p not in sys.path:
            sys.path.insert(0, p)
    import concourse.bass as bass  # noqa: F401

import ml_dtypes
import concourse.bacc as bacc
import concourse.mybir as mybir
import concourse.tile as tile
from concourse.bass import MemorySpace
from concourse.bass_utils import run_bass_kernel_spmd

F32 = mybir.dt.float32
BF16 = mybir.dt.bfloat16
AF = mybir.ActivationFunctionType

N_CORES = 8
N_AGENTS = 16
BATCH = 8192
OBS = 64
HID = 128
N = BATCH * N_AGENTS            # 131072 nodes
NPC = N // N_CORES              # 16384 nodes / core
MC = 4096                       # nodes per macro-chunk
NMC = NPC // MC                 # 8
GPM = MC // N_AGENTS            # 128 graphs per macro
OUTPC = NPC // N_AGENTS         # 1024 graphs per core
S = N_AGENTS
Q = 1024                        # u1/tanh piece size
NQ = MC // Q                    # pieces per macro
DVE_U2 = 1024                   # u2 columns handled by DVE (rest GPSIMD)

_CACHE = {}


def _build_nc():
    nc = bacc.Bacc("TRN2", target_bir_lowering=False, debug=False)

    obs_d = nc.dram_tensor("obs", [NMC, 128, MC], BF16, kind="ExternalInput")
    wcomb_d = nc.dram_tensor("wcomb", [128, HID], BF16, kind="ExternalInput")
    w2_d = nc.dram_tensor("w2s", [HID, HID], BF16, kind="ExternalInput")
    wfc_d = nc.dram_tensor("wfc", [HID, 1], BF16, kind="ExternalInput")
    b1f_d = nc.dram_tensor("b1f", [HID, 1], F32, kind="ExternalInput")
    b2_d = nc.dram_tensor("b2", [HID, 1], F32, kind="ExternalInput")
    out_d = nc.dram_tensor("out", [1, OUTPC], F32, kind="ExternalOutput")

    with tile.TileContext(nc) as tc:
        with (
            tc.tile_pool(name="const", bufs=1) as cp,
            tc.tile_pool(name="zt", bufs=3) as ztp,
            tc.tile_pool(name="sc", bufs=2) as scp,
            tc.tile_pool(name="x1p", bufs=3) as x1p,
            tc.tile_pool(name="u2p", bufs=3) as u2p,
            tc.tile_pool(name="x2p", bufs=3) as x2p,
            tc.tile_pool(name="hsp", bufs=2) as hsp,
            tc.tile_pool(name="hbp", bufs=2) as hbp,
            tc.tile_pool(name="pu1", bufs=2, space=MemorySpace.PSUM) as pu1,
            tc.tile_pool(name="psm", bufs=2, space=MemorySpace.PSUM) as psm,
        ):
            wcomb = cp.tile([128, HID], BF16)
            nc.sync.dma_start(wcomb[:], wcomb_d[:])
            w2 = cp.tile([HID, HID], BF16)
            nc.sync.dma_start(w2[:], w2_d[:])
            wfc = cp.tile([HID, 1], BF16)
            nc.sync.dma_start(wfc[:], wfc_d[:])
            b1f = cp.tile([HID, 1], F32)
            nc.sync.dma_start(b1f[:], b1f_d[:])
            b2 = cp.tile([HID, 1], F32)
            nc.sync.dma_start(b2[:], b2_d[:])
            outsb = cp.tile([1, OUTPC], F32)

            zt_of, u1_of, x1_of, hb_of, u2_of, x2_of = {}, {}, {}, {}, {}, {}

            def stage_a_dma(m):
                zt = ztp.tile([128, MC], BF16, tag="zt")
                zt_of[m] = zt
                nc.sync.dma_start(zt[:], obs_d[m])

            def tree16(src_ap, dst_ap, width, tag, part):
                """Contiguous pairwise s-tree: src [p, 16*width] -> dst [p, width]."""
                a = scp.tile([part, 8 * width], BF16, tag=tag + "a")
                nc.vector.tensor_add(a[:], src_ap[:, 0:8 * width],
                                     src_ap[:, 8 * width:16 * width])
                b = scp.tile([part, 4 * width], BF16, tag=tag + "b")
                nc.vector.tensor_add(b[:], a[:, 0:4 * width], a[:, 4 * width:8 * width])
                c = scp.tile([part, 2 * width], BF16, tag=tag + "c")
                nc.vector.tensor_add(c[:], b[:, 0:2 * width], b[:, 2 * width:4 * width])
                nc.vector.tensor_add(dst_ap, c[:, 0:width], c[:, width:2 * width])

            def stage_a(m):
                if m not in zt_of:
                    stage_a_dma(m)

            def stage_b(m, q):
                zt = zt_of[m]
                u1 = pu1.tile([HID, Q], F32, tag="u1")
                u1_of[(m, q)] = u1
                o = q * Q
                nc.tensor.matmul(u1[:, 0:512], wcomb[:], zt[:, o:o + 512],
                                 start=True, stop=True)
                nc.tensor.matmul(u1[:, 512:1024], wcomb[:], zt[:, o + 512:o + 1024],
                                 start=True, stop=True)
                if q == NQ - 1:
                    zt_of.pop(m)

            def stage_c(m, q):
                u1 = u1_of.pop((m, q))
                if q == 0:
                    x1t = x1p.tile([HID, MC], BF16, tag="x1")
                    x1_of[m] = x1t
                x1 = x1_of[m]
                o = q * Q
                nc.scalar.activation(x1[:, o:o + Q], u1[:], AF.Tanh, bias=b1f[:])

            def stage_d(m):
                x1 = x1_of[m]
                sx1 = scp.tile([HID, GPM], BF16, tag="sx1")
                tree16(x1[:], sx1[:], GPM, "s1", HID)
                h2p = psm.tile([HID, GPM], F32, tag="h2")
                nc.tensor.matmul(h2p[:], w2[:], sx1[:], start=True, stop=True)
                h2s = hsp.tile([HID, GPM], BF16, tag="h2s")
                nc.vector.tensor_scalar_add(h2s[:], h2p[:], b2[:])
                # broadcast h2s to all 16 node slots via SBUF->SBUF DMA
                h2b = hbp.tile([HID, MC], BF16, tag="h2b")
                hb_of[m] = h2b
                src = h2s[:].rearrange("p (o g) -> p o g", o=1)
                nc.scalar.dma_start(
                    h2b[:].rearrange("p (s g) -> p s g", s=16),
                    src.broadcast_to([HID, 16, GPM]),
                )

            def stage_e(m):
                x1 = x1_of.pop(m)
                h2b = hb_of.pop(m)
                u2t = u2p.tile([HID, MC], BF16, tag="u2")
                u2_of[m] = u2t
                half = MC // 2
                nc.vector.tensor_add(u2t[:, 0:half], x1[:, 0:half], h2b[:, 0:half])
                nc.vector.tensor_add(u2t[:, half:MC], x1[:, half:MC], h2b[:, half:MC])

            def stage_f(m, q):
                u2 = u2_of[m]
                if q == 0:
                    x2t = x2p.tile([HID, MC], BF16, tag="x2")
                    x2_of[m] = x2t
                x2 = x2_of[m]
                o = q * Q
                nc.scalar.activation(x2[:, o:o + Q], u2[:, o:o + Q], AF.Tanh)
                if q == NQ - 1:
                    u2_of.pop(m)

            def stage_g(m):
                x2 = x2_of.pop(m)
                sx2 = scp.tile([HID, GPM], BF16, tag="sx2")
                tree16(x2[:], sx2[:], GPM, "s2", HID)
                yp = psm.tile([1, GPM], F32, tag="y")
                nc.tensor.matmul(yp[:], wfc[:], sx2[:], start=True, stop=True)
                nc.vector.tensor_copy(outsb[0:1, m * GPM:(m + 1) * GPM], yp[:])

            # software pipeline over macros: tanh1 a macro ahead, tanh2 and
            # the head a macro behind, so no engine queue waits on the long
            # h2-broadcast chain.
            stage_a(0); stage_a(1); stage_a(2)
            for q in range(NQ):
                stage_b(0, q); stage_c(0, q)
            for m in range(NMC + 1):
                if m + 3 < NMC:
                    stage_a(m + 3)
                if m + 1 < NMC:
                    for q in range(NQ):
                        stage_b(m + 1, q); stage_c(m + 1, q)
                if m < NMC:
                    stage_d(m)
                    stage_e(m)
                if m >= 1:
                    for q in range(NQ):
                        stage_f(m - 1, q)
                    stage_g(m - 1)

            nc.sync.dma_start(out_d[:], outsb[:])

    nc.compile()
    return nc


def _get_nc():
    if "nc" not in _CACHE:
        _CACHE["nc"] = _build_nc()
    return _CACHE["nc"]


def _make_in_maps(cent_obs, w_emb, b_emb, w_gcn, b_gcn, w_fc1):
    w_emb = np.ascontiguousarray(w_emb, np.float32)
    wcomb = np.concatenate(
        [w_emb, (w_emb @ w_gcn[0]) / np.float32(16.0)], axis=0
    ).astype(ml_dtypes.bfloat16)
    w2s = (w_gcn[1] / np.float32(16.0)).astype(ml_dtypes.bfloat16)
    wfc = (w_fc1.reshape(HID, 1) / np.float32(16.0)).astype(ml_dtypes.bfloat16)
    b1f = (b_gcn[0] + b_emb + b_emb @ w_gcn[0]).astype(np.float32).reshape(HID, 1)
    b2 = b_gcn[1].astype(np.float32).reshape(HID, 1)
    shared = {"wcomb": wcomb, "w2s": w2s, "wfc": wfc, "b1f": b1f, "b2": b2}
    o5 = np.ascontiguousarray(cent_obs, np.float32).reshape(
        N_CORES, NMC, GPM, S, OBS
    )
    # node n = core*NPC + m*MC + g*16 + s  ->  obs_dev[core, m, 0:64, s*GPM+g];
    # rows 64:128 carry the per-graph obs sums, pre-broadcast to all 16 slots
    top = o5.transpose(0, 1, 4, 3, 2)                      # [C, M, OBS, S, GPM]
    sob = o5.sum(axis=3).transpose(0, 1, 3, 2)             # [C, M, OBS, GPM]
    bot = np.broadcast_to(sob[:, :, :, None, :], top.shape)
    obs_all = np.concatenate([top, bot], axis=2).astype(ml_dtypes.bfloat16)
    obs_all = obs_all.reshape(N_CORES, NMC, 128, MC)
    in_maps = []
    for ci in range(N_CORES):
        m = dict(shared)
        m["obs"] = np.ascontiguousarray(obs_all[ci])
        in_maps.append(m)
    return in_maps


def kernel(cent_obs, w_emb, b_emb, w_gcn, b_gcn, w_fc1, b_fc1,
           edge_src, edge_dst, _trace=False):
    cent_obs = np.asarray(cent_obs, np.float32)
    nc = _get_nc()
    in_maps = _make_in_maps(
        cent_obs, np.asarray(w_emb, np.float32), np.asarray(b_emb, np.float32),
        np.asarray(w_gcn, np.float32), np.asarray(b_gcn, np.float32),
        np.asarray(w_fc1, np.float32),
    )
    kw = dict(trace=True) if _trace else {}
    res = run_bass_kernel_spmd(nc, in_maps, list(range(N_CORES)), **kw)
    y = np.concatenate(
        [np.asarray(res.results[i]["out"]).reshape(-1) for i in range(N_CORES)]
    )
    out = (y + np.float32(np.asarray(b_fc1).reshape(()))).astype(np.float32)
    if _trace:
        _CACHE["last_result"] = res
    return out.reshape(BATCH, 1)
